# revision 2
# baseline (speedup 1.0000x reference)
"""Trainium2 Bass kernel for CubeFaceNN.

Computes, for x of shape [8, 1, 128, 128, 128] (f32):
    out[b, i, p] = relu(x[b, 0, p] - x[b, 0, p + OFF[i]])   (zero padded)
with OFF = [(0,-1,-1), (-1,0,-1), (1,-1,-1), (-1,1,-1), (-1,-1,0), (-1,-1,1)]
(derived from the reference's adj % 3 - 1 indexing).

Sharding: pure data parallel - batch b -> NeuronCore b (8 cores).

Design (v2): all data preparation moved to DRAM padding; zero on-chip
data marshalling.
  - Host passes xpad (fp16, [129, 128, 128]) = [zero plane, x]. Two
    full-128-partition loads give both operands:
      xt16 = xpad[1:129]  (xt16[d] = x[d])
      xs16 = xpad[0:128]  (xs16[d] = x[d-1], row 0 = zero padding)
    so the d-axis shift costs no PE matmul / PSUM copies, and the fp16
    input halves the load bytes and removes the ACT cast pass.
  - Output DRAM is padded [6, 129, H, W]: channel i lives in planes
    [i, 1:129]. ch2 (od=+1) is computed in the substituted frame
    och[d'] = out[2, d'-1] = relu(xs16[d'] - xt16[d', h-1, w-1]) on all
    128 partitions (partition 0 is garbage) and stored to planes
    [2, 0:128] - partition 0 lands in the trash plane. Every store is
    therefore a full-128-partition HWDGE ring DMA (127-partition ring
    DMAs degenerate to serial descriptor processing, and SWDGE is ~2x
    slower). The missing out[2,127] = relu(x[127]) plane is patched
    from a small [h, w]-layout tile.
  - Channels uniformly: och = relu(A - B[shifted by delta]) + boundary
    strips relu(A rows/cols) where the shifted source is zero padding.
  - Engine split: DVE does 20 subs + ch0/ch2 relus; GpSimd does ch4's
    sub+relu; ACT does ch1/3/5 relus + strips; sync ring carries all
    stores + xt loads, scalar ring carries xs loads (triggers on the
    two HWDGE-capable engines only; sync runs no compute so waiting
    triggers block nothing).
"""

import numpy as np

import concourse.bacc as bacc
import concourse.mybir as mybir
import concourse.tile as tile
from concourse.bass_utils import run_bass_kernel_spmd

D = H = W = 128
HW = H * W
UH = 32  # unit = h-quarter
UF = UH * W
NU = H // UH
N_CORES = 8
F16 = mybir.dt.float16

# channel spec: (A, B, delta, oh, ow) -- och = relu(A - B<<delta) with
# strips relu(A) on the h/w boundary rows/cols given by oh/ow. A/B in
# {"t": xt16, "s": xs16}. ch2 is the substituted (d' = d+1) frame.
CH_SPEC = [
    ("t", "t", -(W + 1), -1, -1),  # ch0 (0,-1,-1)
    ("t", "s", -1, 0, -1),         # ch1 (-1,0,-1)
    ("s", "t", -(W + 1), -1, -1),  # ch2 (1,-1,-1) substituted
    ("t", "s", W - 1, 1, -1),      # ch3 (-1,1,-1)
    ("t", "s", -W, -1, 0),         # ch4 (-1,-1,0)
    ("t", "s", -W + 1, -1, 1),     # ch5 (-1,-1,1)
]

# engine per channel for sub+relu: v=DVE, a=DVE sub + ACT relu, g=GpSimd
CH_ENG = ["v", "a", "v", "a", "g", "a"]
# wave-internal emission order: lead with the two DVE-relu channels so
# stores start early, interleave ACT/GpSimd channels behind them
SUB_ORDER = (0, 2, 1, 4, 3, 5)

# load row chunks (both xt and xs): wave u needs rows [32u-2, 32u+33]
LOAD_ROWS = [0, 34, 66, 98, 128]

_NC_CACHE = {}


def build_nc(debug=False):
    nc = bacc.Bacc("TRN2", target_bir_lowering=False, debug=debug)
    xpad = nc.dram_tensor("xpad", [D + 1, H, W], F16, kind="ExternalInput")
    outp = nc.dram_tensor("outp", [6, D + 1, H, W], F16, kind="ExternalOutput")

    sub = mybir.AluOpType.subtract
    relu = mybir.ActivationFunctionType.Relu

    with tile.TileContext(nc) as tc:
        with (
            tc.tile_pool(name="xt16", bufs=1) as xt_pool,
            tc.tile_pool(name="xs16", bufs=1) as xs_pool,
            tc.tile_pool(name="och", bufs=9) as och_pool,
            tc.tile_pool(name="pf16", bufs=2) as pf_pool,
        ):
            xt16 = xt_pool.tile([D, H, W], F16)
            xs16 = xs_pool.tile([D, H, W], F16)
            xt2 = xt16.rearrange("d h w -> d (h w)")
            xs2 = xs16.rearrange("d h w -> d (h w)")
            AB = {"t": (xt16, xt2), "s": (xs16, xs2)}

            # loads: xt chunks on the sync ring, xs chunks on the scalar
            # ring (scalar's 4 trigger slots run before its compute)
            for c in range(4):
                hsl = slice(LOAD_ROWS[c], LOAD_ROWS[c + 1])
                nc.sync.dma_start(out=xt16[:, hsl], in_=xpad[1 : D + 1, hsl])
                nc.scalar.dma_start(out=xs16[:, hsl], in_=xpad[0:D, hsl])

            def emit_unit(i, u):
                A3, A2 = AB[CH_SPEC[i][0]]
                _, B2 = AB[CH_SPEC[i][1]]
                delta, oh, ow = CH_SPEC[i][2:]
                eng = CH_ENG[i]
                och = och_pool.tile([D, UH, W], F16, name="och")
                och2 = och.rearrange("d h w -> d (h w)")

                f0, f1 = u * UF, (u + 1) * UF
                lo = max(f0, -delta)
                hi = min(f1, HW - delta)
                r0 = u * UH

                sub_eng = nc.gpsimd if eng == "g" else nc.vector
                sub_eng.tensor_tensor(
                    out=och2[:, lo - f0 : hi - f0],
                    in0=A2[:, lo:hi],
                    in1=B2[:, lo + delta : hi + delta],
                    op=sub,
                )
                # interior relu in place, right behind the sub
                osel = och2[:, lo - f0 : hi - f0]
                if eng == "v":
                    nc.vector.tensor_scalar_max(osel, osel, 0.0)
                elif eng == "g":
                    nc.gpsimd.tensor_scalar_max(osel, osel, 0.0)
                else:
                    nc.scalar.activation(osel, osel, relu)

                # boundary strips: shifted source is zero padding -> relu(A)
                def strip(osel_, asel_):
                    if eng == "v":
                        nc.vector.tensor_scalar_max(och[osel_], A3[asel_], 0.0)
                    elif eng == "g":
                        nc.gpsimd.tensor_scalar_max(och[osel_], A3[asel_], 0.0)
                    else:
                        nc.scalar.activation(och[osel_], A3[asel_], relu)

                if oh == -1 and u == 0:
                    strip((slice(0, D), slice(0, 1)), (slice(0, D), slice(0, 1)))
                if oh == 1 and u == NU - 1:
                    strip(
                        (slice(0, D), slice(UH - 1, UH)),
                        (slice(0, D), slice(H - 1, H)),
                    )
                if ow != 0:
                    wb = 0 if ow == -1 else W - 1
                    hs, he = max(0, -oh), H - max(0, oh)
                    rs, re = max(hs, r0), min(he, r0 + UH)
                    strip(
                        (slice(0, D), slice(rs - r0, re - r0), slice(wb, wb + 1)),
                        (slice(0, D), slice(rs, re), slice(wb, wb + 1)),
                    )

                # store: full-128-partition ring DMA on sync. ch2 targets
                # planes [0:128] (partition 0 -> trash plane), the rest
                # planes [1:129].
                p0 = 0 if i == 2 else 1
                nc.sync.dma_start(
                    out=outp[i, p0 : p0 + D, r0 : r0 + UH], in_=och[:]
                )

            for u in range(NU):
                for i in SUB_ORDER:
                    emit_unit(i, u)
                if u == 0:
                    # patch plane out[2,127] = relu(x[127]) ([h, w] layout)
                    p1 = pf_pool.tile([H, W], F16)
                    p1r = pf_pool.tile([H, W], F16)
                    nc.scalar.dma_start(out=p1[:], in_=xpad[D])
                    nc.scalar.activation(p1r[:], p1[:], relu)
                    nc.scalar.dma_start(out=outp[2, D], in_=p1r[:])

    nc.compile()
    return nc


def _get_nc():
    if "nc" not in _NC_CACHE:
        _NC_CACHE["nc"] = build_nc()
    return _NC_CACHE["nc"]


def prep_input(xb: np.ndarray) -> np.ndarray:
    """[D, H, W] f32 -> padded fp16 [D+1, H, W] with zero plane 0."""
    xp = np.empty((D + 1, H, W), dtype=np.float16)
    xp[0] = 0
    xp[1:] = xb
    return xp


def kernel(x: np.ndarray) -> np.ndarray:
    assert x.shape == (N_CORES, 1, D, H, W), x.shape
    nc = _get_nc()
    in_maps = [{"xpad": prep_input(x[b, 0])} for b in range(N_CORES)]
    res = run_bass_kernel_spmd(nc, in_maps, core_ids=list(range(N_CORES)))
    return np.stack(
        [np.asarray(r["outp"])[:, 1:].astype(np.float32) for r in res.results],
        axis=0,
    )


# revision 3
# speedup vs baseline: 3.1859x; 3.1859x over previous
"""Trainium2 Bass kernel for CubeFaceNN.

Computes, for x of shape [8, 1, 128, 128, 128] (f32):
    out[b, i, p] = relu(x[b, 0, p] - x[b, 0, p + OFF[i]])   (zero padded)
with OFF = [(0,-1,-1), (-1,0,-1), (1,-1,-1), (-1,1,-1), (-1,-1,0), (-1,-1,1)]
(derived from the reference's adj % 3 - 1 indexing).

Sharding: pure data parallel - batch b -> NeuronCore b (8 cores).

Design (v2): all data preparation moved to DRAM padding; zero on-chip
data marshalling.
  - Host passes xpad (fp16, [129, 128, 128]) = [zero plane, x]. Two
    full-128-partition loads give both operands:
      xt16 = xpad[1:129]  (xt16[d] = x[d])
      xs16 = xpad[0:128]  (xs16[d] = x[d-1], row 0 = zero padding)
    so the d-axis shift costs no PE matmul / PSUM copies, and the fp16
    input halves the load bytes and removes the ACT cast pass.
  - Output DRAM is padded [6, 129, H, W]: channel i lives in planes
    [i, 1:129]. ch2 (od=+1) is computed in the substituted frame
    och[d'] = out[2, d'-1] = relu(xs16[d'] - xt16[d', h-1, w-1]) on all
    128 partitions (partition 0 is garbage) and stored to planes
    [2, 0:128] - partition 0 lands in the trash plane. Every store is
    therefore a full-128-partition HWDGE ring DMA (127-partition ring
    DMAs degenerate to serial descriptor processing, and SWDGE is ~2x
    slower). The missing out[2,127] = relu(x[127]) plane is patched
    from a small [h, w]-layout tile.
  - Channels uniformly: och = relu(A - B[shifted by delta]) + boundary
    strips relu(A rows/cols) where the shifted source is zero padding.
  - Engine split: DVE does 20 subs + ch0/ch2 relus; GpSimd does ch4's
    sub+relu; ACT does ch1/3/5 relus + strips; sync ring carries all
    stores + xt loads, scalar ring carries xs loads (triggers on the
    two HWDGE-capable engines only; sync runs no compute so waiting
    triggers block nothing).
"""

import numpy as np

import concourse.bacc as bacc
import concourse.mybir as mybir
import concourse.tile as tile
from concourse.bass_utils import run_bass_kernel_spmd

D = H = W = 128
HW = H * W
UH = 32  # unit = h-quarter
UF = UH * W
NU = H // UH
N_CORES = 8
F16 = mybir.dt.float16

# channel spec: (A, B, delta, oh, ow) -- och = relu(A - B<<delta) with
# strips relu(A) on the h/w boundary rows/cols given by oh/ow. A/B in
# {"t": xt16, "s": xs16}. ch2 is the substituted (d' = d+1) frame.
CH_SPEC = [
    ("t", "t", -(W + 1), -1, -1),  # ch0 (0,-1,-1)
    ("t", "s", -1, 0, -1),         # ch1 (-1,0,-1)
    ("s", "t", -(W + 1), -1, -1),  # ch2 (1,-1,-1) substituted
    ("t", "s", W - 1, 1, -1),      # ch3 (-1,1,-1)
    ("t", "s", -W, -1, 0),         # ch4 (-1,-1,0)
    ("t", "s", -W + 1, -1, 1),     # ch5 (-1,-1,1)
]

# engine per channel for sub+relu: v=DVE, a=DVE sub + ACT relu, g=GpSimd
# (GpSimd measured ~40x slower than DVE per element -- never use it)
CH_ENG = ["v", "a", "v", "a", "v", "a"]
# wave-internal emission order: lead with the two DVE-relu channels so
# stores start early, interleave ACT/GpSimd channels behind them
SUB_ORDER = (0, 2, 1, 4, 3, 5)

# load row chunks (both xt and xs): wave u needs rows [32u-2, 32u+33]
LOAD_ROWS = [0, 34, 66, 98, 128]

_NC_CACHE = {}


def build_nc(debug=False):
    nc = bacc.Bacc("TRN2", target_bir_lowering=False, debug=debug)
    xpad = nc.dram_tensor("xpad", [D + 1, H, W], F16, kind="ExternalInput")
    outp = nc.dram_tensor("outp", [6, D + 1, H, W], F16, kind="ExternalOutput")

    sub = mybir.AluOpType.subtract
    relu = mybir.ActivationFunctionType.Relu

    with tile.TileContext(nc) as tc:
        with (
            tc.tile_pool(name="xt16", bufs=1) as xt_pool,
            tc.tile_pool(name="xs16", bufs=1) as xs_pool,
            tc.tile_pool(name="och", bufs=9) as och_pool,
            tc.tile_pool(name="pf16", bufs=2) as pf_pool,
        ):
            xt16 = xt_pool.tile([D, H, W], F16)
            xs16 = xs_pool.tile([D, H, W], F16)
            xt2 = xt16.rearrange("d h w -> d (h w)")
            xs2 = xs16.rearrange("d h w -> d (h w)")
            AB = {"t": (xt16, xt2), "s": (xs16, xs2)}

            # loads: xt chunks on the sync ring, xs chunks on the scalar
            # ring (scalar's 4 trigger slots run before its compute)
            for c in range(4):
                hsl = slice(LOAD_ROWS[c], LOAD_ROWS[c + 1])
                nc.sync.dma_start(out=xt16[:, hsl], in_=xpad[1 : D + 1, hsl])
                nc.scalar.dma_start(out=xs16[:, hsl], in_=xpad[0:D, hsl])

            def emit_unit(i, u):
                A3, A2 = AB[CH_SPEC[i][0]]
                _, B2 = AB[CH_SPEC[i][1]]
                delta, oh, ow = CH_SPEC[i][2:]
                eng = CH_ENG[i]
                och = och_pool.tile([D, UH, W], F16, name="och")
                och2 = och.rearrange("d h w -> d (h w)")

                f0, f1 = u * UF, (u + 1) * UF
                lo = max(f0, -delta)
                hi = min(f1, HW - delta)
                r0 = u * UH

                sub_eng = nc.gpsimd if eng == "g" else nc.vector
                sub_eng.tensor_tensor(
                    out=och2[:, lo - f0 : hi - f0],
                    in0=A2[:, lo:hi],
                    in1=B2[:, lo + delta : hi + delta],
                    op=sub,
                )
                # interior relu in place, right behind the sub
                osel = och2[:, lo - f0 : hi - f0]
                if eng == "v":
                    nc.vector.tensor_scalar_max(osel, osel, 0.0)
                elif eng == "g":
                    nc.gpsimd.tensor_scalar_max(osel, osel, 0.0)
                else:
                    nc.scalar.activation(osel, osel, relu)

                # boundary strips: shifted source is zero padding -> relu(A)
                def strip(osel_, asel_):
                    if eng == "v":
                        nc.vector.tensor_scalar_max(och[osel_], A3[asel_], 0.0)
                    elif eng == "g":
                        nc.gpsimd.tensor_scalar_max(och[osel_], A3[asel_], 0.0)
                    else:
                        nc.scalar.activation(och[osel_], A3[asel_], relu)

                if oh == -1 and u == 0:
                    strip((slice(0, D), slice(0, 1)), (slice(0, D), slice(0, 1)))
                if oh == 1 and u == NU - 1:
                    strip(
                        (slice(0, D), slice(UH - 1, UH)),
                        (slice(0, D), slice(H - 1, H)),
                    )
                if ow != 0:
                    wb = 0 if ow == -1 else W - 1
                    hs, he = max(0, -oh), H - max(0, oh)
                    rs, re = max(hs, r0), min(he, r0 + UH)
                    strip(
                        (slice(0, D), slice(rs - r0, re - r0), slice(wb, wb + 1)),
                        (slice(0, D), slice(rs, re), slice(wb, wb + 1)),
                    )

                # store: full-128-partition ring DMA on sync. ch2 targets
                # planes [0:128] (partition 0 -> trash plane), the rest
                # planes [1:129].
                p0 = 0 if i == 2 else 1
                nc.sync.dma_start(
                    out=outp[i, p0 : p0 + D, r0 : r0 + UH], in_=och[:]
                )

            for u in range(NU):
                for i in SUB_ORDER:
                    emit_unit(i, u)
                if u == 0:
                    # patch plane out[2,127] = relu(x[127]) ([h, w] layout)
                    p1 = pf_pool.tile([H, W], F16)
                    p1r = pf_pool.tile([H, W], F16)
                    nc.scalar.dma_start(out=p1[:], in_=xpad[D])
                    nc.scalar.activation(p1r[:], p1[:], relu)
                    nc.scalar.dma_start(out=outp[2, D], in_=p1r[:])

    nc.compile()
    return nc


def _get_nc():
    if "nc" not in _NC_CACHE:
        _NC_CACHE["nc"] = build_nc()
    return _NC_CACHE["nc"]


def prep_input(xb: np.ndarray) -> np.ndarray:
    """[D, H, W] f32 -> padded fp16 [D+1, H, W] with zero plane 0."""
    xp = np.empty((D + 1, H, W), dtype=np.float16)
    xp[0] = 0
    xp[1:] = xb
    return xp


def kernel(x: np.ndarray) -> np.ndarray:
    assert x.shape == (N_CORES, 1, D, H, W), x.shape
    nc = _get_nc()
    in_maps = [{"xpad": prep_input(x[b, 0])} for b in range(N_CORES)]
    res = run_bass_kernel_spmd(nc, in_maps, core_ids=list(range(N_CORES)))
    return np.stack(
        [np.asarray(r["outp"])[:, 1:].astype(np.float32) for r in res.results],
        axis=0,
    )


# revision 4
# speedup vs baseline: 3.4262x; 1.0754x over previous
"""Trainium2 Bass kernel for CubeFaceNN.

Computes, for x of shape [8, 1, 128, 128, 128] (f32):
    out[b, i, p] = relu(x[b, 0, p] - x[b, 0, p + OFF[i]])   (zero padded)
with OFF = [(0,-1,-1), (-1,0,-1), (1,-1,-1), (-1,1,-1), (-1,-1,0), (-1,-1,1)]
(derived from the reference's adj % 3 - 1 indexing).

Sharding: pure data parallel - batch b -> NeuronCore b (8 cores).

Design (v3): minimize DMA bytes; the slowest of the 16 round-robin DMA
engines (~21.5 GB/s vs 25.8 for its peers, static per-packet
round-robin) is the critical path, so total bytes is the only DMA lever.
  - Host sends x as fp16 [128, 128, 128] (the 2e-2 max-norm gate admits
    fp16 rounding, ~6e-4). One 4.2 MB load -> xt16; the depth-shifted
    operand xs16[d] = xt16[d-1] (plane 0 = zero padding) is generated
    on-chip by the PE with a one-subdiagonal one-hot shift matrix
    (values exact in fp16) and drained PSUM->SBUF by ACT copies, 512
    f32 per PSUM bank chunk.
  - Output DRAM is padded [6, 129, H, W]: channel i lives in planes
    [i, 1:129]. ch2 (od=+1) is computed in the substituted frame
    och[d'] = out[2, d'-1] = relu(xs16[d'] - xt16[d', h-1, w-1]) on all
    128 partitions (partition 0 is garbage) and stored to planes
    [2, 0:128] - partition 0 lands in the trash plane. Every store is
    a full-128-partition HWDGE ring DMA (127-partition ring DMAs
    degenerate; SWDGE/GpSimd is ~40x slower per element for compute and
    ~2x for DMA). out[2,127] = relu(x[127]) is patched from a small
    [h, w]-layout tile.
  - Channels uniformly: och = relu(A - B<<delta) + boundary strips
    relu(A rows/cols) where the shifted source is zero padding.
  - Engine budget (measured rates: DVE sub 2.27us, DVE relu 1.23us,
    ACT relu 4.3us per [128, 4096] fp16 unit; ACT PSUM copy 0.55us per
    512-chunk): DVE = 24 subs + 14 relus ~= 74us; ACT = 10 relus + 32
    copies + strips + p1 ~= 73us; both under the ~86us DMA critical
    path. Stores ride both rings: sync for DVE-relu'd units, scalar
    (right after the relu, zero wait) for ACT-relu'd units.
"""

import numpy as np

import concourse.bacc as bacc
import concourse.mybir as mybir
import concourse.tile as tile
from concourse.bass_utils import run_bass_kernel_spmd

D = H = W = 128
HW = H * W
UH = 32  # unit = h-quarter
UF = UH * W
NU = H // UH
N_CORES = 8
MMF = 512  # matmul moving free size (one PSUM bank of f32)
NCHUNK = HW // MMF
F32 = mybir.dt.float32
F16 = mybir.dt.float16

# channel spec: (A, B, delta, oh, ow) -- och = relu(A - B<<delta) with
# strips relu(A) on the h/w boundary rows/cols given by oh/ow. A/B in
# {"t": xt16, "s": xs16}. ch2 is the substituted (d' = d+1) frame.
CH_SPEC = [
    ("t", "t", -(W + 1), -1, -1),  # ch0 (0,-1,-1)
    ("t", "s", -1, 0, -1),         # ch1 (-1,0,-1)
    ("s", "t", -(W + 1), -1, -1),  # ch2 (1,-1,-1) substituted
    ("t", "s", W - 1, 1, -1),      # ch3 (-1,1,-1)
    ("t", "s", -W, -1, 0),         # ch4 (-1,-1,0)
    ("t", "s", -W + 1, -1, 1),     # ch5 (-1,-1,1)
]

# relu engine per (channel, wave): v=DVE tensor_scalar_max, a=ACT
# activation. 14 DVE / 10 ACT balances DVE ~74us vs ACT ~73us.
RELU_ENG = {
    0: "vvvv",
    1: "aaaa",
    2: "vvvv",
    3: "aaaa",
    4: "vvvv",
    5: "avav",
}
# wave-internal emission order: xt-only ch0 first (its sub needs no xs
# copies), then the other DVE-relu'd channels, ACT channels behind
SUB_ORDER = (0, 2, 1, 4, 3, 5)

# load row chunks, aligned to 512-flat (4-row) matmul chunks; wave u
# needs rows [32u-2, 32u+33]
LOAD_ROWS = [0, 36, 68, 100, 128]

_NC_CACHE = {}


def build_nc(debug=False):
    nc = bacc.Bacc("TRN2", target_bir_lowering=False, debug=debug)
    x16 = nc.dram_tensor("x16", [D, H, W], F16, kind="ExternalInput")
    outp = nc.dram_tensor("outp", [6, D + 1, H, W], F16, kind="ExternalOutput")
    # shift matrix: sh[k, m] = 1 iff k == m-1, so (sh.T @ v)[m] = v[m-1]
    # (column 0 all-zero -> xs16[0] = 0, the zero padding at d = -1)
    sh_dram = nc.inline_tensor(np.eye(D, k=1, dtype=np.float16), name="shift")

    sub = mybir.AluOpType.subtract
    relu = mybir.ActivationFunctionType.Relu

    with tile.TileContext(nc) as tc:
        with (
            tc.tile_pool(name="xt16", bufs=1) as xt_pool,
            tc.tile_pool(name="xs16", bufs=1) as xs_pool,
            tc.tile_pool(name="sh", bufs=1) as sh_pool,
            tc.tile_pool(name="och", bufs=9) as och_pool,
            tc.tile_pool(name="pf16", bufs=2) as pf_pool,
            tc.tile_pool(name="ps", bufs=8, space="PSUM") as ps_pool,
        ):
            sht = sh_pool.tile([D, D], F16)
            nc.sync.dma_start(out=sht[:], in_=sh_dram[:])

            xt16 = xt_pool.tile([D, H, W], F16)
            xs16 = xs_pool.tile([D, H, W], F16)
            xt2 = xt16.rearrange("d h w -> d (h w)")
            xs2 = xs16.rearrange("d h w -> d (h w)")
            AB = {"t": (xt16, xt2), "s": (xs16, xs2)}

            for c in range(4):
                hsl = slice(LOAD_ROWS[c], LOAD_ROWS[c + 1])
                nc.sync.dma_start(out=xt16[:, hsl], in_=x16[:, hsl])

            # xs16 = PE shift of xt16, chunked by PSUM bank; ACT drains.
            # Program order IS Tile's hazard order: every chunk must be
            # emitted before its consumers.
            copy_next = 0

            def emit_copies(upto):
                nonlocal copy_next
                for k in range(copy_next, min(NCHUNK, upto)):
                    ps = ps_pool.tile([D, MMF], F32)
                    nc.tensor.matmul(
                        out=ps[:],
                        lhsT=sht[:],
                        rhs=xt2[:, k * MMF : (k + 1) * MMF],
                        start=True,
                        stop=True,
                    )
                    nc.scalar.copy(out=xs2[:, k * MMF : (k + 1) * MMF], in_=ps[:])
                copy_next = max(copy_next, min(NCHUNK, upto))

            def emit_unit(i, u):
                A3, A2 = AB[CH_SPEC[i][0]]
                _, B2 = AB[CH_SPEC[i][1]]
                delta, oh, ow = CH_SPEC[i][2:]
                eng = RELU_ENG[i][u]
                och = och_pool.tile([D, UH, W], F16, name="och")
                och2 = och.rearrange("d h w -> d (h w)")

                f0, f1 = u * UF, (u + 1) * UF
                lo = max(f0, -delta)
                hi = min(f1, HW - delta)
                r0 = u * UH

                nc.vector.tensor_tensor(
                    out=och2[:, lo - f0 : hi - f0],
                    in0=A2[:, lo:hi],
                    in1=B2[:, lo + delta : hi + delta],
                    op=sub,
                )
                # interior relu in place
                osel = och2[:, lo - f0 : hi - f0]
                if eng == "v":
                    nc.vector.tensor_scalar_max(osel, osel, 0.0)
                else:
                    nc.scalar.activation(osel, osel, relu)

                # boundary strips: shifted source is zero padding -> relu(A)
                def strip(osel_, asel_):
                    if eng == "v":
                        nc.vector.tensor_scalar_max(och[osel_], A3[asel_], 0.0)
                    else:
                        nc.scalar.activation(och[osel_], A3[asel_], relu)

                if oh == -1 and u == 0:
                    strip((slice(0, D), slice(0, 1)), (slice(0, D), slice(0, 1)))
                if oh == 1 and u == NU - 1:
                    strip(
                        (slice(0, D), slice(UH - 1, UH)),
                        (slice(0, D), slice(H - 1, H)),
                    )
                if ow != 0:
                    wb = 0 if ow == -1 else W - 1
                    hs, he = max(0, -oh), H - max(0, oh)
                    rs, re = max(hs, r0), min(he, r0 + UH)
                    strip(
                        (slice(0, D), slice(rs - r0, re - r0), slice(wb, wb + 1)),
                        (slice(0, D), slice(rs, re), slice(wb, wb + 1)),
                    )

                # store: full-128-partition ring DMA. ch2 targets planes
                # [0:128] (partition 0 -> trash plane), the rest [1:129].
                # ACT-relu'd units trigger on the scalar ring right after
                # their relu (zero wait); DVE-relu'd on the sync ring.
                p0 = 0 if i == 2 else 1
                ring = nc.sync if eng == "v" else nc.scalar
                ring.dma_start(out=outp[i, p0 : p0 + D, r0 : r0 + UH], in_=och[:])

            for u in range(NU):
                for j, i in enumerate(SUB_ORDER):
                    if j == 1:
                        # chunks wave u's xs consumers read: flat window
                        # [4096u - 257, 4096(u+1) + 127]
                        emit_copies(9 + 8 * u)
                    emit_unit(i, u)
                emit_copies(17 + 8 * u)
                if u == 0:
                    # patch plane out[2,127] = relu(x[127]) ([h, w] layout)
                    p1 = pf_pool.tile([H, W], F16)
                    p1r = pf_pool.tile([H, W], F16)
                    nc.scalar.dma_start(out=p1[:], in_=x16[D - 1])
                    nc.scalar.activation(p1r[:], p1[:], relu)
                    nc.scalar.dma_start(out=outp[2, D], in_=p1r[:])

    nc.compile()
    return nc


def _get_nc():
    if "nc" not in _NC_CACHE:
        _NC_CACHE["nc"] = build_nc()
    return _NC_CACHE["nc"]


def prep_input(xb: np.ndarray) -> np.ndarray:
    """[D, H, W] f32 -> fp16."""
    return np.asarray(xb, dtype=np.float16)


def kernel(x: np.ndarray) -> np.ndarray:
    assert x.shape == (N_CORES, 1, D, H, W), x.shape
    nc = _get_nc()
    in_maps = [{"x16": prep_input(x[b, 0])} for b in range(N_CORES)]
    res = run_bass_kernel_spmd(nc, in_maps, core_ids=list(range(N_CORES)))
    return np.stack(
        [np.asarray(r["outp"])[:, 1:].astype(np.float32) for r in res.results],
        axis=0,
    )
